# revision 1
# baseline (speedup 1.0000x reference)
"""Deformable 3x3 conv (torchvision offset layout), N=8,C=O=256,H=W=64,
stride=1,pad=1,dil=1 on 8 NeuronCores, data-parallel over batch.

Per-core pipeline (one image):
  - cast x to fp16, PE-transpose to x_t[HW+1, C] in DRAM (gather source)
  - offsets -> bilinear corner weights (f32) + pair-gather row indices (int16
    tables in the dma_gather 16-partition wrap, built via double PE-transpose)
  - dma_gather row-pairs (2 corners / descriptor) into [sample, 2*C] tiles
  - DVE tensor_scalar/scalar_tensor_tensor chain applies the 4 corner weights
  - PE-transpose sampled[s,c] tiles into matmul rhs layout [ck, s]
  - fp16 matmul (fp32 PSUM accum) with W[ck,o], bias add on evacuation
"""
import numpy as np

import concourse.bacc as bacc
import concourse.mybir as mybir
import concourse.tile as tile
from concourse import bass_utils
from concourse.bass import AP

F16 = mybir.dt.float16
F32 = mybir.dt.float32
I16 = mybir.dt.int16
I32 = mybir.dt.int32
ALU = mybir.AluOpType
ACTF = mybir.ActivationFunctionType

N, C, O, H, W, KS = 8, 256, 256, 64, 64, 3
K = KS * KS
S = H * W            # 4096 output samples (stride 1)
QC = S // 128        # 32 q columns; sample s lives at [p=s//QC, q=s%QC]
R = 4                # regions
QR = QC // R         # 8 q per region
ROWS = S + 1
KQ = K * QC

_CACHE = {}


def _build(repeat=1):
    nc = bacc.Bacc("TRN2", target_bir_lowering=False, debug=False,
                   enable_asserts=True, num_devices=8)
    xin = nc.dram_tensor("xin", [C, S], F32, kind="ExternalInput")
    off = nc.dram_tensor("off", [2 * K, S], F32, kind="ExternalInput")
    win = nc.dram_tensor("win", [O, C * K], F32, kind="ExternalInput")
    bin_ = nc.dram_tensor("bin", [O], F32, kind="ExternalInput")
    by8 = nc.dram_tensor("by8", [128, KQ], F32, kind="ExternalInput")
    bx8 = nc.dram_tensor("bx8", [128, KQ], F32, kind="ExternalInput")
    id16 = nc.dram_tensor("id16", [128, 128], F16, kind="ExternalInput")
    id32 = nc.dram_tensor("id32", [128, 128], F32, kind="ExternalInput")
    xt = nc.dram_tensor("xt", [ROWS * C], F16, kind="Internal")
    out = nc.dram_tensor("out", [O, S], F32, kind="ExternalOutput")

    with tile.TileContext(nc) as tc:
        with tc.tile_pool(name="const", bufs=1) as cp:
            i16 = cp.tile([128, 128], F16)
            nc.sync.dma_start(i16[:, :], id16[:, :])
            i32 = cp.tile([128, 128], F32)
            nc.sync.dma_start(i32[:, :], id32[:, :])
            bias_sb = cp.tile([128, 2], F32)
            nc.sync.dma_start(bias_sb[:, :], AP(bin_, 0, [[1, 128], [128, 2]]))
            wt = cp.tile([128, 2 * K, 256], F16)
            tableR = cp.tile([128, 2 * K * 256], I16)
            w00 = cp.tile([128, KQ], F32)
            w01 = cp.tile([128, KQ], F32)
            w10 = cp.tile([128, KQ], F32)
            w11 = cp.tile([128, KQ], F32)
            osb = cp.tile([128, 2, S], F32)

            # ================= prep phase (pools freed afterwards) ========
            with tc.tile_pool(name="prep", bufs=1) as pp, \
                 tc.tile_pool(name="prept", bufs=4) as mp0, \
                 tc.tile_pool(name="psA", bufs=2, space="PSUM") as psA:

                # ---- weights: W[o, c*9+k] -> wt[(c%128), 2k+ch, o] fp16
                w_sb = pp.tile([128, 2, C * K], F32, tag="w_sb")
                nc.sync.dma_start(
                    w_sb[:, :, :],
                    AP(win, 0, [[C * K, 128], [128 * C * K, 2], [1, C * K]]))
                for k in range(K):
                    for chc in range(2):
                        pw = psA.tile([128, 256], F32, tag="ps")
                        for och in range(2):
                            src = w_sb[:, och, :].rearrange(
                                "P (c k) -> P k c", k=K)[:, k, chc * 128:(chc + 1) * 128]
                            nc.tensor.transpose(
                                pw[:, och * 128:(och + 1) * 128], src, i32[:, :])
                        nc.vector.tensor_copy(wt[:, 2 * k + chc, :], pw[:, :])

                # ---- x -> fp16 -> x_t[row, c] in DRAM
                x16 = pp.tile([128, 2, S], F16, tag="x16")
                nc.gpsimd.dma_start(
                    x16[:, :, :], AP(xin, 0, [[S, 128], [128 * S, 2], [1, S]]))
                for j in range(32):
                    px = psA.tile([128, 2, 128], F16, tag="ps")
                    for ch in range(2):
                        nc.tensor.transpose(
                            px[:, ch, :], x16[:, ch, j * 128:(j + 1) * 128], i16[:, :])
                    xts = mp0.tile([128, 256], F16, tag="xts")
                    nc.vector.tensor_copy(
                        xts[:, :], px[:, :, :].rearrange("P a b -> P (a b)"))
                    nc.sync.dma_start(
                        AP(xt, j * 128 * C, [[C, 128], [1, 256]]), xts[:, :])

                # ---- offsets -> corner weights + flat indices
                def f32t(tag):
                    return pp.tile([128, KQ], F32, tag=tag, name=tag)

                offy = pp.tile([128, K, QC], F32, tag="offy")
                nc.sync.dma_start(offy[:, :, :],
                                  AP(off, 0, [[QC, 128], [2 * S, K], [1, QC]]))
                offx = pp.tile([128, K, QC], F32, tag="offx")
                nc.sync.dma_start(offx[:, :, :],
                                  AP(off, S, [[QC, 128], [2 * S, K], [1, QC]]))
                byt = f32t("byt")
                nc.sync.dma_start(byt[:, :], by8[:, :])
                bxt = f32t("bxt")
                nc.sync.dma_start(bxt[:, :], bx8[:, :])

                def floor8(pos8, tg):
                    ii = pp.tile([128, KQ], I32, tag=tg + "i", name=tg + "i")
                    nc.vector.tensor_copy(ii[:, :], pos8[:, :])
                    rr = f32t(tg + "r")
                    nc.vector.tensor_copy(rr[:, :], ii[:, :])
                    mm = f32t(tg + "m")
                    nc.vector.tensor_tensor(mm[:, :], rr[:, :], pos8[:, :], ALU.is_gt)
                    ff = f32t(tg + "f")
                    nc.vector.tensor_tensor(ff[:, :], rr[:, :], mm[:, :], ALU.subtract)
                    return ff

                def ts2(dst, src, s1, s2, o1, o2):
                    nc.vector.tensor_scalar(out=dst[:, :], in0=src[:, :], scalar1=s1,
                                            scalar2=s2, op0=o1, op1=o2)

                def tt(dst, a, b, op):
                    nc.vector.tensor_tensor(dst[:, :], a[:, :], b[:, :], op)

                py8 = f32t("py8")
                nc.vector.tensor_tensor(
                    py8[:, :], offy[:, :, :].rearrange("P a b -> P (a b)"),
                    byt[:, :], ALU.add)
                y0f = floor8(py8, "y0")
                fy = f32t("fy")
                tt(fy, py8, y0f, ALU.subtract)
                y0c = f32t("y0c")
                ts2(y0c, y0f, 8.0, 71.0, ALU.max, ALU.min)
                y1f = f32t("y1f")
                nc.vector.tensor_scalar(out=y1f[:, :], in0=y0f[:, :], scalar1=1.0,
                                        scalar2=None, op0=ALU.add)
                y1c = f32t("y1c")
                ts2(y1c, y1f, 8.0, 71.0, ALU.max, ALU.min)

                def valid(v, tg):
                    a = f32t(tg + "a")
                    nc.vector.tensor_scalar(out=a[:, :], in0=v[:, :], scalar1=8.0,
                                            scalar2=None, op0=ALU.is_ge)
                    b = f32t(tg + "b")
                    nc.vector.tensor_scalar(out=b[:, :], in0=v[:, :], scalar1=71.0,
                                            scalar2=None, op0=ALU.is_le)
                    m = f32t(tg + "v")
                    tt(m, a, b, ALU.mult)
                    return m

                vy0 = valid(y0f, "vy0")
                vy1 = valid(y1f, "vy1")
                fy1 = f32t("fy1")
                ts2(fy1, fy, -1.0, 1.0, ALU.mult, ALU.add)
                wy0 = f32t("wy0")
                tt(wy0, fy1, vy0, ALU.mult)
                wy1 = f32t("wy1")
                tt(wy1, fy, vy1, ALU.mult)

                px8 = f32t("px8")
                nc.vector.tensor_tensor(
                    px8[:, :], offx[:, :, :].rearrange("P a b -> P (a b)"),
                    bxt[:, :], ALU.add)
                x0f = floor8(px8, "x0")
                fx = f32t("fx")
                tt(fx, px8, x0f, ALU.subtract)
                xb8 = f32t("xb8")
                ts2(xb8, x0f, 8.0, 70.0, ALU.max, ALU.min)
                dd = f32t("dd")
                tt(dd, x0f, xb8, ALU.subtract)
                eqs = []
                for tg, val in (("e0", 0.0), ("em", -1.0), ("ep", 1.0)):
                    e = f32t(tg)
                    nc.vector.tensor_scalar(out=e[:, :], in0=dd[:, :], scalar1=val,
                                            scalar2=None, op0=ALU.is_equal)
                    eqs.append(e)
                e0, em, ep = eqs
                fx1 = f32t("fx1")
                ts2(fx1, fx, -1.0, 1.0, ALU.mult, ALU.add)
                pw0 = f32t("pw0")
                t1 = f32t("pt1")
                tt(t1, fx1, e0, ALU.mult)
                t2 = f32t("pt2")
                tt(t2, fx, em, ALU.mult)
                tt(pw0, t1, t2, ALU.add)
                pw1 = f32t("pw1")
                t3 = f32t("pt3")
                tt(t3, fx, e0, ALU.mult)
                t4 = f32t("pt4")
                tt(t4, fx1, ep, ALU.mult)
                tt(pw1, t3, t4, ALU.add)

                tt(w00, wy0, pw0, ALU.mult)
                tt(w01, wy0, pw1, ALU.mult)
                tt(w10, wy1, pw0, ALU.mult)
                tt(w11, wy1, pw1, ALU.mult)

                # flat = yc*64 + xb - 520 (both corners stacked on fl axis)
                flS = pp.tile([128, K, 2, QC], F32, tag="flS")
                for fl, yc in ((0, y0c), (1, y1c)):
                    nc.vector.scalar_tensor_tensor(
                        out=flS[:, :, fl, :], in0=yc[:, :].rearrange("P (k q) -> P k q", k=K),
                        scalar=64.0,
                        in1=xb8[:, :].rearrange("P (k q) -> P k q", k=K),
                        op0=ALU.mult, op1=ALU.add)
                    nc.vector.tensor_scalar(
                        out=flS[:, :, fl, :], in0=flS[:, :, fl, :], scalar1=-520.0,
                        scalar2=None, op0=ALU.add)

                # ---- idx tables: fold [128, QC] -> [16, 8*QC] per (k, fl)
                # tableR[p16, (2k+fl)*256 + r*64 + ql*8 + ph] = flS[ph*16+p16, fl, k*QC+q]
                for k in range(K):
                    pa = psA.tile([64, 128], F32, tag="ps")
                    nc.tensor.transpose(
                        pa[:, :],
                        flS[:, k, :, :].rearrange("P a b -> P (a b)"),
                        i32[:, :])
                    asb = mp0.tile([64, 128], F32, tag="asb")
                    nc.vector.tensor_copy(asb[:, :], pa[:, :])
                    pt = psA.tile([16, 8, 64], F32, tag="ps2")
                    for ph in range(8):
                        nc.tensor.transpose(
                            pt[:, ph, :], asb[:, ph * 16:(ph + 1) * 16],
                            i32[0:64, 0:64])
                    dst = tableR[0:16, 2 * k * 256:2 * k * 256 + 512].rearrange(
                        "P (fl r q ph) -> P ph fl r q", fl=2, r=R, q=QR)
                    src = pt[:, :, :].rearrange("P a (fl r q) -> P a fl r q", fl=2, r=R)
                    nc.vector.tensor_copy(dst, src)
                for g in range(1, 8):
                    nc.sync.dma_start(tableR[g * 16:(g + 1) * 16, :], tableR[0:16, :])

            # ================= main loop =================
            with tc.tile_pool(name="gpool", bufs=2) as gp, \
                 tc.tile_pool(name="spool", bufs=1) as sp, \
                 tc.tile_pool(name="tmp", bufs=4) as mp, \
                 tc.tile_pool(name="psT", bufs=2, space="PSUM") as psT, \
                 tc.tile_pool(name="psC", bufs=2, space="PSUM") as psC:
                in_ap = AP(xt, 0, [[256, ROWS - 2], [1, 512]])
                for rep in range(repeat):
                    for r in range(R):
                        sam = sp.tile([128, 2 * K, 1024], F16, tag="sam")
                        for k in range(K):
                            g0 = gp.tile([128, QR, 512], F16, tag="g0")
                            g1 = gp.tile([128, QR, 512], F16, tag="g1")
                            for fl, gt in ((0, g0), (1, g1)):
                                base = (2 * k + fl) * 256 + r * 64
                                nc.gpsimd.dma_gather(
                                    out_ap=gt[:, :, :], in_ap=in_ap,
                                    idxs_ap=tableR[:, base:base + 64],
                                    num_idxs=128 * QR, num_idxs_reg=128 * QR,
                                    elem_size=512, elem_step=256,
                                    single_packet=False)
                            ptt = psT.tile([128, 2, QR, 128], F16, tag="ptt")
                            for ql in range(QR):
                                col = k * QC + r * QR + ql
                                tt0 = mp.tile([128, 256], F16, tag="tt0")
                                nc.vector.tensor_scalar(
                                    out=tt0[:, :], in0=g0[:, ql, 0:256],
                                    scalar1=w00[:, col:col + 1], scalar2=None,
                                    op0=ALU.mult)
                                tt1 = mp.tile([128, 256], F16, tag="tt1")
                                nc.vector.scalar_tensor_tensor(
                                    out=tt1[:, :], in0=g0[:, ql, 256:512],
                                    scalar=w01[:, col:col + 1], in1=tt0[:, :],
                                    op0=ALU.mult, op1=ALU.add)
                                tt2 = mp.tile([128, 256], F16, tag="tt2")
                                nc.vector.scalar_tensor_tensor(
                                    out=tt2[:, :], in0=g1[:, ql, 0:256],
                                    scalar=w10[:, col:col + 1], in1=tt1[:, :],
                                    op0=ALU.mult, op1=ALU.add)
                                tt3 = mp.tile([128, 256], F16, tag="tt3")
                                nc.vector.scalar_tensor_tensor(
                                    out=tt3[:, :], in0=g1[:, ql, 256:512],
                                    scalar=w11[:, col:col + 1], in1=tt2[:, :],
                                    op0=ALU.mult, op1=ALU.add)
                                for ch in range(2):
                                    nc.tensor.transpose(
                                        ptt[:, ch, ql, :],
                                        tt3[:, ch * 128:(ch + 1) * 128], i16[:, :])
                            for ch in range(2):
                                nc.vector.tensor_copy(
                                    sam[:, 2 * k + ch, :].rearrange(
                                        "P (a b) -> P a b", a=QR),
                                    ptt[:, ch, :, :])
                        # conv for region r
                        for st in range(2):
                            for och in range(2):
                                pc = psC.tile([128, 512], F32, tag="pc")
                                for ck in range(2 * K):
                                    nc.tensor.matmul(
                                        pc[:, :],
                                        wt[:, ck, och * 128:(och + 1) * 128],
                                        sam[:, ck, st * 512:(st + 1) * 512],
                                        start=(ck == 0), stop=(ck == 2 * K - 1))
                                if rep == 0:
                                    dst = osb[:, och, :].rearrange(
                                        "P (pc s) -> P s pc", s=QC)[
                                        :, r * QR + st * 4:r * QR + st * 4 + 4, :]
                                else:
                                    scr = mp.tile([128, 4, 128], F32, tag="oscr")
                                    dst = scr[:, :, :]
                                nc.vector.tensor_scalar(
                                    out=dst,
                                    in0=pc[:, :].rearrange("P (a b) -> P a b", a=4),
                                    scalar1=bias_sb[:, och:och + 1], scalar2=None,
                                    op0=ALU.add)
                nc.sync.dma_start(
                    AP(out, 0, [[S, 128], [128 * S, 2], [1, S]]), osb[:, :, :])

    nc.compile()
    return nc


def _consts():
    p = np.arange(128)[:, None].astype(np.float32)
    q = np.arange(QC)[None, :].astype(np.float32)
    by = np.zeros((128, KQ), np.float32)
    bx = np.zeros((128, KQ), np.float32)
    for k in range(K):
        by[:, k * QC:(k + 1) * QC] = np.floor(p / 2) - 1 + (k // KS) + 8
        bx[:, k * QC:(k + 1) * QC] = np.mod(p, 2) * 32 + q - 1 + (k % KS) + 8
    return by, bx


def _in_maps(x, offset, weight, bias):
    by, bx = _consts()
    id16 = np.eye(128, dtype=np.float16)
    id32 = np.eye(128, dtype=np.float32)
    x = np.ascontiguousarray(np.asarray(x, np.float32).reshape(N, C, S))
    offset = np.ascontiguousarray(np.asarray(offset, np.float32).reshape(N, 2 * K, S))
    wf = np.ascontiguousarray(np.asarray(weight, np.float32).reshape(O, C * K))
    bf = np.ascontiguousarray(np.asarray(bias, np.float32).reshape(O))
    return [{"xin": x[i], "off": offset[i], "win": wf, "bin": bf,
             "by8": by, "bx8": bx, "id16": id16, "id32": id32} for i in range(N)]


def kernel(x, offset, weight, bias, stride):
    stride = int(np.asarray(stride))
    assert stride == 1, "only stride=1 supported"
    if "nc" not in _CACHE:
        _CACHE["nc"] = _build()
    nc = _CACHE["nc"]
    res = bass_utils.run_bass_kernel_spmd(nc, _in_maps(x, offset, weight, bias),
                                          core_ids=list(range(8)))
    outs = np.stack([res.results[i]["out"] for i in range(N)])
    return outs.reshape(N, O, H, W).astype(np.float32)

